# revision 49
# baseline (speedup 1.0000x reference)
"""CosAttn2d (cosFormer-style 2D linear attention) Trainium2 kernel, v2.

Problem (hardcoded): q,k,v [32, 512, 32, 32] f32, n_head=8, M=48.
Sharding: data-parallel over batch across 8 cores (4 batches each).

v2 redesign vs v1 (177us -> 99us in the TimelineSim cost model):
- separable coef factors: y-factor as per-partition ptr TS on the k side
  (dn folded in, eps dropped: 1e-5 is 4-5 orders below the tolerance;
  q-side eps kept - D hits exact 0 on these inputs without it),
  x-factor as per-chunk ptr TS on the v side
- ksum folded into the ctx matmul via host-appended ones columns in vsp
  (psum col 128 of each 130-wide var block = branch ksum)
- wide-N ctx streams: 2 MMs of N=260 per (pair, chunk) instead of 8
- bulk elementwise balanced across DVE/ACT/Pool (v1 ran the 16 qm
  multiplies on GPSIMD at 0.5 elem/cyc = its 93%-busy bottleneck)
- one consolidated store DMA per batch; tiny consts DMA'd before the 1MB
  inputs (the DMA device serializes transfers); normalize TTs deferred
  past the next batch's front so DVE never blocks on the dbc DMA chain;
  batch 0's k/v transfers split into token-halves (qnb between them) so
  the first ctx MMs start ~6us earlier; D-path sps/dps recycle one PSUM
  bank, freeing a 5th ctx accumulation bank

Branch order j = 2*stat + var: [cc, sc, cs, ss] where stat 0/1 = y-factor
cos/sin (on k), var 0/1 = x-factor cos/sin (on v). Host reorders coef
tables to match.
"""

import sys

sys.path.insert(0, "/opt/trn_rl_repo")

from contextlib import ExitStack
from math import pi

import numpy as np
import ml_dtypes

from concourse import bass, mybir, tile
from concourse.bass_utils import run_bass_kernel_spmd

F32 = mybir.dt.float32
BF16 = mybir.dt.bfloat16
BFNP = ml_dtypes.bfloat16

B, C, H, W = 32, 512, 32, 32
NHEAD, MVAL = 8, 48
N = H * W            # 1024 tokens
NCORES = 8
BPC = B // NCORES    # batches per core
DN = float(C) ** -0.25
EPS = 1e-5

MULT = mybir.AluOpType.mult
ADD = mybir.AluOpType.add
MAX = mybir.AluOpType.max
COPY = mybir.ActivationFunctionType.Copy


def _fix_waits(nc):
    """This walrus build rejects sync waits on CTRL_NO-struct instructions
    (Drain/NoOp) and allows at most one on DMACopy. Hoist the excess into
    standalone InstEventSemaphore instructions on the same engine (the
    sequencer executes them in order, so semantics are unchanged)."""
    for fn in nc.m.functions:
        for bb in fn.blocks:
            newlist = []
            for inst in bb.instructions:
                si = inst.sync_info
                if isinstance(inst, mybir.InstEventSemaphore):
                    cap = 1
                elif isinstance(inst, (mybir.InstDrain, mybir.InstNoOp)):
                    cap = 0
                else:
                    cap = 1
                if si is not None and len(si.on_wait) > cap:
                    waits = list(si.on_wait)
                    for w in waits[cap:]:
                        ev = mybir.InstEventSemaphore(
                            name=nc.get_next_instruction_name(), ins=[], outs=[])
                        ev.engine = inst.engine
                        ev.sync_info = mybir.SyncInfo(on_wait=[w], on_update=[])
                        nc.register_instruction(ev, overwrite=True)
                        newlist.append(ev)
                    inst.sync_info = mybir.SyncInfo(
                        on_wait=waits[:cap], on_update=list(si.on_update))
                newlist.append(inst)
            bb.instructions[:] = newlist


def build_nc():
    nc = bass.Bass()

    qn_d = nc.declare_dram_parameter("qn", [BPC, C, N], BF16, isOutput=False)
    ks_d = nc.declare_dram_parameter("ks", [BPC, N, C], BF16, isOutput=False)
    vsp_d = nc.declare_dram_parameter("vsp", [BPC, N, 4, 130], BF16, isOutput=False)
    ptrs_d = nc.declare_dram_parameter("ptrs", [128, 4], F32, isOutput=False)
    cx_d = nc.declare_dram_parameter("cx", [128, 8], F32, isOutput=False)
    sx_d = nc.declare_dram_parameter("sx", [128, 8], F32, isOutput=False)
    coefq_d = nc.declare_dram_parameter("coefq", [128, 4, N], BF16, isOutput=False)
    coefD_d = nc.declare_dram_parameter("coefD", [128, N], BF16, isOutput=False)
    dcomb_d = nc.declare_dram_parameter("dcomb", [128, 32], BF16, isOutput=False)
    o_d = nc.declare_dram_parameter("o", [BPC, C, N], BF16, isOutput=True)

    with tile.TileContext(nc) as tc, ExitStack() as ctx:
        const = ctx.enter_context(tc.tile_pool(name="const", bufs=1))
        stat = ctx.enter_context(tc.tile_pool(name="stat", bufs=1))
        io = ctx.enter_context(tc.tile_pool(name="io", bufs=2))
        wk1 = ctx.enter_context(tc.tile_pool(name="wk1", bufs=1))
        wk2 = ctx.enter_context(tc.tile_pool(name="wk2", bufs=2))
        qmp = ctx.enter_context(tc.tile_pool(name="qmp", bufs=1))
        small = ctx.enter_context(tc.tile_pool(name="small", bufs=2))
        dbcp = ctx.enter_context(tc.tile_pool(name="dbcp", bufs=1))
        ocp = ctx.enter_context(tc.tile_pool(name="ocp", bufs=1))
        pctx = ctx.enter_context(tc.tile_pool(name="pctx", bufs=5, space="PSUM"))
        pout = ctx.enter_context(tc.tile_pool(name="pout", bufs=2, space="PSUM"))
        psd = ctx.enter_context(tc.tile_pool(name="psd", bufs=1, space="PSUM"))
        dram = ctx.enter_context(tc.tile_pool(name="dram", bufs=2, space="DRAM"))

        def issue_inputs(b, split=False):
            ksr = io.tile([128, 8, C], BF16, tag="ksr", name="ksr")
            qnb = io.tile([128, 4, N], BF16, tag="qnb", name="qnb")
            vst = io.tile([128, 8, 4, 130], BF16, tag="vst", name="vst")
            if split:
                # batch 0: half-granularity k/v transfers so the first ctx
                # MMs start as soon as the first token chunks land
                def khalf(h):
                    ts = slice(512 * h, 512 * (h + 1))
                    nc.sync.dma_start(
                        ksr[:, 4 * h:4 * h + 4, :],
                        ks_d[b, ts].rearrange("(s r) c -> r s c", r=128))
                    nc.sync.dma_start(
                        vst[:, 4 * h:4 * h + 4],
                        vsp_d[b, ts].rearrange("(s r) p c -> r s p c", r=128))
                khalf(0)
                nc.sync.dma_start(
                    qnb[:], qn_d[b].rearrange("(p r) n -> r p n", r=128))
                khalf(1)
            else:
                nc.sync.dma_start(
                    ksr[:], ks_d[b].rearrange("(s r) c -> r s c", r=128))
                nc.sync.dma_start(
                    qnb[:], qn_d[b].rearrange("(p r) n -> r p n", r=128))
                nc.sync.dma_start(
                    vst[:], vsp_d[b].rearrange("(s r) p c -> r s p c", r=128))
            return qnb, ksr, vst

        # tiny constants first (they gate the k/v-side elementwise ops and
        # the serialized DMA device would otherwise park them behind 3MB of
        # input transfers); big coef tables after the first batch's inputs
        ptrs = const.tile([128, 4], F32, tag="ptrs", name="ptrs")
        cx = const.tile([128, 8], F32, tag="cx", name="cx")
        sx = const.tile([128, 8], F32, tag="sx", name="sx")
        dcomb = const.tile([128, 32], BF16, tag="dcomb", name="dcomb")
        nc.sync.dma_start(ptrs[:], ptrs_d[:])
        nc.sync.dma_start(cx[:], cx_d[:])
        nc.sync.dma_start(sx[:], sx_d[:])
        nc.sync.dma_start(dcomb[:], dcomb_d[:])

        first_in = issue_inputs(0, split=True)

        coefq = const.tile([128, 4, N], BF16, tag="coefq", name="coefq")
        coefD = const.tile([128, N], BF16, tag="coefD", name="coefD")
        nc.sync.dma_start(coefq[:], coefq_d[:])
        nc.sync.dma_start(coefD[:], coefD_d[:])

        # persistent zero-initialized tiles (written regions rewritten per
        # batch, zero regions never touched); double-buffered by parity
        L = {}
        kD = {}
        for par in range(2):
            for p in range(4):
                Lt = stat.tile([128, 2, 2, 128], BF16, tag=f"L{par}{p}",
                               name=f"L{par}{p}")
                nc.gpsimd.memset(Lt[:].rearrange("r a b e -> r (a b e)"), 0.0)
                L[par, p] = Lt
                kt = stat.tile([128, 32], BF16, tag=f"kD{par}{p}",
                               name=f"kD{par}{p}")
                nc.gpsimd.memset(kt[:], 0.0)
                kD[par, p] = kt

        def front(b, qnb, ksr, vst, split=False):
            """k/q-side elementwise prep for batch b. Ordered so Pool's qm
            work (needs qa) can start as early as possible."""
            # eps terms dropped: 1e-5 is 4-5 orders below bf16 noise and the
            # 2e-2 gate; dn folded into the cosy/siny ptrs
            kc = wk1.tile([128, 8, C], BF16, tag="kc", name="kc")
            ks2 = wk1.tile([128, 8, C], BF16, tag="ks2", name="ks2")
            for sh in ((slice(0, 4), slice(4, 8)) if split else (slice(0, 8),)):
                nc.vector.tensor_scalar(
                    kc[:, sh, :].rearrange("r s c -> r (s c)"),
                    ksr[:, sh, :].rearrange("r s c -> r (s c)"),
                    ptrs[:, 0:1], 0.0, MULT, MAX)
                nc.vector.tensor_scalar(
                    ks2[:, sh, :].rearrange("r s c -> r (s c)"),
                    ksr[:, sh, :].rearrange("r s c -> r (s c)"),
                    ptrs[:, 1:2], 0.0, MULT, MAX)
            qr = wk1.tile([128, 4, N], BF16, tag="qr", name="qr")
            nc.vector.tensor_scalar(
                qr[:].rearrange("r p n -> r (p n)"),
                qnb[:].rearrange("r p n -> r (p n)"), DN, 0.0, MULT, MAX)
            qa = wk2.tile([128, 4, N], BF16, tag="qa", name="qa")
            for p in range(4):
                nc.scalar.activation(qa[:, p, :], qr[:, p, :], COPY, bias=EPS)
            vCS = wk1.tile([128, 8, 4, 2, 130], BF16, tag="vCS", name="vCS")
            for s in range(8):
                nc.vector.tensor_scalar_mul(
                    vCS[:, s, :, 0, :], vst[:, s], cx[:, s:s + 1])
                nc.vector.tensor_scalar_mul(
                    vCS[:, s, :, 1, :], vst[:, s], sx[:, s:s + 1])
            return kc, ks2, qa, vCS

        def norm_and_store(st, split=False):
            b, ocs, dbc, onrm = st
            for p in range(4):
                for half in range(2):
                    hs = slice(512 * half, 512 * (half + 1))
                    nc.vector.tensor_tensor(
                        onrm[:, p, hs], ocs[p, half][:], dbc[p][:, hs], MULT)
                if split:
                    nc.sync.dma_start(
                        o_d[b, 128 * p:128 * (p + 1), :], onrm[:, p, :])
            if not split:
                nc.sync.dma_start(
                    o_d[b].rearrange("(p r) n -> r p n", r=128), onrm[:])

        fr = front(0, *first_in, split=True)
        pending = None
        for b in range(BPC):
            par = b % 2
            kc, ks2, qa, vCS = fr
            if b + 1 < BPC:
                nxt_in = issue_inputs(b + 1)
            if pending is not None:
                norm_and_store(pending)
                pending = None

            # ---- qm_j = qa * coefq_j (DVE planes / Pool planes) ----
            qm = {}
            for j in range(4):
                t = qmp.tile([128, 4, N], BF16, tag=f"qm{j}", name=f"qm{j}")
                qm[j] = t
                ndve = 3 if j <= 1 else 2
                cb = coefq[:, j, :]
                nc.vector.tensor_tensor(
                    t[:, 0:ndve, :],
                    qa[:, 0:ndve, :],
                    cb.unsqueeze(1).broadcast_to([128, ndve, N]), MULT)
                nc.gpsimd.tensor_tensor(
                    t[:, ndve:4, :],
                    qa[:, ndve:4, :],
                    cb.unsqueeze(1).broadcast_to([128, 4 - ndve, N]), MULT)

            # ---- ctx per pair: 2 psum tiles, 16 MMs; harvest L + kD ----
            for p in range(4):
                pA = pctx.tile([128, 2, 130], F32, tag="pc", name="pA")
                pB = pctx.tile([128, 2, 130], F32, tag="pc", name="pB")
                for s in range(8):
                    nc.tensor.matmul(
                        pA[:], kc[:, s, 128 * p:128 * (p + 1)], vCS[:, s, p],
                        start=(s == 0), stop=(s == 7))
                    nc.tensor.matmul(
                        pB[:], ks2[:, s, 128 * p:128 * (p + 1)], vCS[:, s, p],
                        start=(s == 0), stop=(s == 7))
                for st, pX in ((0, pA), (1, pB)):
                    for h in range(2):
                        rs = slice(64 * h, 64 * (h + 1))
                        nc.scalar.activation(
                            L[par, p][rs, st, :, rs], pX[rs, :, rs], COPY)
                        nc.scalar.activation(
                            kD[par, p][rs, 2 * st + 4 * h:2 * st + 4 * h + 2],
                            pX[rs, :, 128:129], COPY)

            # ---- D: sps = kD^T qa ; sd = sps*coefD ; dps = dcomb^T sd ----
            dinv = small.tile([8, N], BF16, tag="dinv", name="dinv")
            for half in range(2):
                hs = slice(512 * half, 512 * (half + 1))
                sps = psd.tile([128, 512], F32, tag="sps", name="sps")
                for p in range(4):
                    nc.tensor.matmul(
                        sps[32 * p:32 * (p + 1), :], kD[par, p][:],
                        qa[:, p, hs], start=True, stop=True,
                        tile_position=(0, 32 * p))
                sd = small.tile([128, 512], BF16, tag="sd", name="sd")
                nc.vector.tensor_tensor(sd[:], sps[:], coefD[:, hs], MULT)
                dps = psd.tile([32, 512], F32, tag="sps", name="dps")
                nc.tensor.matmul(dps[:], dcomb[:], sd[:], start=True, stop=True)
                with nc.allow_low_precision(reason="bf16 D_inv within tolerance"):
                    nc.vector.reciprocal(dinv[0:8, hs], dps[0:8, :])

            # broadcast D_inv rows to 64 partitions each via DRAM bounce
            dvd = dram.tile([8, N], BF16, tag="dvd", name="dvd")
            nc.sync.dma_start(dvd[:], dinv[:])
            dbc = {}
            for p in range(4):
                dbc[p] = dbcp.tile([128, N], BF16, tag=f"dbc{p}", name=f"dbc{p}")
                nc.sync.dma_start(
                    dbc[p][:],
                    dvd[2 * p:2 * p + 2, :]
                    .partition_broadcast(64).rearrange("r h n -> h r n"))

            # ---- out phase; PSUM drained promptly by ACT copies. The
            # normalize TTs are deferred past the next batch's front so the
            # DVE queue never blocks on the dbc DMA chain. ----
            onrm = small.tile([128, 4, N], BF16, tag="onrm", name="onrm")
            ocs = {}
            for p in range(4):
                for half in range(2):
                    hs = slice(512 * half, 512 * (half + 1))
                    ops = pout.tile([128, 512], F32, tag="ops", name="ops")
                    for j in range(4):
                        st, var = j // 2, j % 2
                        nc.tensor.matmul(
                            ops[:], L[par, p][:, st, var, :], qm[j][:, p, hs],
                            start=(j == 0), stop=(j == 3))
                    oc = ocp.tile([128, 512], BF16, tag=f"oc{p}{half}",
                                  name="oc")
                    nc.scalar.activation(oc[:], ops[:], COPY)
                    ocs[p, half] = oc

            if b + 1 < BPC:
                fr = front(b + 1, *nxt_in)
            pending = (b, ocs, dbc, onrm)
        norm_and_store(pending, split=True)

    _fix_waits(nc)
    return nc


_NC = None


def _get_nc():
    global _NC
    if _NC is None:
        _NC = build_nc()
    return _NC


def _host_prep(q, k, v, n_head, M):
    n_head = int(n_head)
    M = int(M)
    assert q.shape == (B, C, H, W) and n_head == NHEAD

    idx = np.arange(H, dtype=np.float32)
    freq = np.float32(pi / (2 * M))
    co, si = np.cos(idx * freq), np.sin(idx * freq)
    t = np.arange(N)
    x, y = t // 32, t % 32
    r = np.arange(128)

    dn32 = np.float32(DN)
    ptrs = np.stack([dn32 * co[r % 32], dn32 * si[r % 32],
                     0 * co[r % 32], 0 * si[r % 32]], axis=1).astype(np.float32)
    cx = np.stack([co[(128 * s + r) // 32] for s in range(8)],
                  axis=1).astype(np.float32)
    sx = np.stack([si[(128 * s + r) // 32] for s in range(8)],
                  axis=1).astype(np.float32)

    # branch order j: [cc, sc, cs, ss]
    coefB = np.stack([co[x] * co[y], si[x] * co[y],
                      co[x] * si[y], si[x] * si[y]]).astype(np.float32)
    coefq = np.broadcast_to(coefB[None], (128, 4, N)).astype(BFNP)
    coefD = np.zeros((128, N), np.float32)
    dcomb = np.zeros((128, 32), np.float32)
    for p in range(4):
        for h in range(2):
            for j in range(4):
                coefD[32 * p + 4 * h + j] = coefB[j]
                dcomb[32 * p + 4 * h + j, 2 * p + h] = 1.0
    coefD = coefD.astype(BFNP)
    dcomb = dcomb.astype(BFNP)

    qf = q.reshape(B, C, N).astype(BFNP)
    kf = np.ascontiguousarray(
        k.reshape(B, C, N).transpose(0, 2, 1)).astype(BFNP)
    vt = np.ascontiguousarray(
        v.reshape(B, C, N).transpose(0, 2, 1))          # [B, N, C] f32
    vsp = np.zeros((B, N, 4, 130), np.float32)
    vsp[..., :128] = vt.reshape(B, N, 4, 128)
    vsp[..., 128] = 1.0
    vsp = vsp.astype(BFNP)

    in_maps = []
    for core in range(NCORES):
        b0 = core * BPC
        in_maps.append({
            "qn": qf[b0:b0 + BPC], "ks": kf[b0:b0 + BPC],
            "vsp": vsp[b0:b0 + BPC],
            "ptrs": ptrs, "cx": cx, "sx": sx,
            "coefq": coefq, "coefD": coefD, "dcomb": dcomb,
        })
    return in_maps


def run(q, k, v, n_head=8, M=48, trace=False):
    nc = _get_nc()
    in_maps = _host_prep(q, k, v, n_head, M)
    res = run_bass_kernel_spmd(nc, in_maps, core_ids=list(range(NCORES)),
                               trace=trace)
    outs = []
    for core in range(NCORES):
        o = np.asarray(res.results[core]["o"]).astype(np.float32)
        outs.append(o.reshape(BPC, C, H, W))
    return np.concatenate(outs, axis=0), res


def kernel(q, k, v, n_head=8, M=48):
    out, _ = run(q, k, v, n_head, M)
    return out


# revision 50
# speedup vs baseline: 1.0010x; 1.0010x over previous
"""CosAttn2d (cosFormer-style 2D linear attention) Trainium2 kernel, v2.

Problem (hardcoded): q,k,v [32, 512, 32, 32] f32, n_head=8, M=48.
Sharding: data-parallel over batch across 8 cores (4 batches each).

v2 redesign vs v1 (177us -> 99us in the TimelineSim cost model):
- separable coef factors: y-factor as per-partition ptr TS on the k side
  (dn folded in, eps dropped: 1e-5 is 4-5 orders below the tolerance;
  q-side eps kept - D hits exact 0 on these inputs without it),
  x-factor as per-chunk ptr TS on the v side
- ksum folded into the ctx matmul via host-appended ones columns in vsp
  (psum col 128 of each 130-wide var block = branch ksum)
- wide-N ctx streams: 2 MMs of N=260 per (pair, chunk) instead of 8
- bulk elementwise balanced across DVE/ACT/Pool (v1 ran the 16 qm
  multiplies on GPSIMD at 0.5 elem/cyc = its 93%-busy bottleneck)
- one consolidated store DMA per batch; tiny consts DMA'd before the 1MB
  inputs (the DMA device serializes transfers); normalize TTs deferred
  past the next batch's front so DVE never blocks on the dbc DMA chain;
  batch 0's k/v transfers split into token-halves (qnb between them) so
  the first ctx MMs start ~6us earlier; D-path sps/dps recycle one PSUM
  bank, freeing a 5th ctx accumulation bank

Branch order j = 2*stat + var: [cc, sc, cs, ss] where stat 0/1 = y-factor
cos/sin (on k), var 0/1 = x-factor cos/sin (on v). Host reorders coef
tables to match.
"""

import sys

sys.path.insert(0, "/opt/trn_rl_repo")

from contextlib import ExitStack
from math import pi

import numpy as np
import ml_dtypes

from concourse import bass, mybir, tile
from concourse.bass_utils import run_bass_kernel_spmd

F32 = mybir.dt.float32
BF16 = mybir.dt.bfloat16
BFNP = ml_dtypes.bfloat16

B, C, H, W = 32, 512, 32, 32
NHEAD, MVAL = 8, 48
N = H * W            # 1024 tokens
NCORES = 8
BPC = B // NCORES    # batches per core
DN = float(C) ** -0.25
EPS = 1e-5

MULT = mybir.AluOpType.mult
ADD = mybir.AluOpType.add
MAX = mybir.AluOpType.max
COPY = mybir.ActivationFunctionType.Copy


def _fix_waits(nc):
    """This walrus build rejects sync waits on CTRL_NO-struct instructions
    (Drain/NoOp) and allows at most one on DMACopy. Hoist the excess into
    standalone InstEventSemaphore instructions on the same engine (the
    sequencer executes them in order, so semantics are unchanged)."""
    for fn in nc.m.functions:
        for bb in fn.blocks:
            newlist = []
            for inst in bb.instructions:
                si = inst.sync_info
                if isinstance(inst, mybir.InstEventSemaphore):
                    cap = 1
                elif isinstance(inst, (mybir.InstDrain, mybir.InstNoOp)):
                    cap = 0
                else:
                    cap = 1
                if si is not None and len(si.on_wait) > cap:
                    waits = list(si.on_wait)
                    for w in waits[cap:]:
                        ev = mybir.InstEventSemaphore(
                            name=nc.get_next_instruction_name(), ins=[], outs=[])
                        ev.engine = inst.engine
                        ev.sync_info = mybir.SyncInfo(on_wait=[w], on_update=[])
                        nc.register_instruction(ev, overwrite=True)
                        newlist.append(ev)
                    inst.sync_info = mybir.SyncInfo(
                        on_wait=waits[:cap], on_update=list(si.on_update))
                newlist.append(inst)
            bb.instructions[:] = newlist


def build_nc():
    nc = bass.Bass()

    qn_d = nc.declare_dram_parameter("qn", [BPC, C, N], BF16, isOutput=False)
    ks_d = nc.declare_dram_parameter("ks", [BPC, N, C], BF16, isOutput=False)
    vsp_d = nc.declare_dram_parameter("vsp", [BPC, N, 4, 130], BF16, isOutput=False)
    ptrs_d = nc.declare_dram_parameter("ptrs", [128, 4], F32, isOutput=False)
    cx_d = nc.declare_dram_parameter("cx", [128, 8], F32, isOutput=False)
    sx_d = nc.declare_dram_parameter("sx", [128, 8], F32, isOutput=False)
    coefq_d = nc.declare_dram_parameter("coefq", [128, 4, N], BF16, isOutput=False)
    coefD_d = nc.declare_dram_parameter("coefD", [128, N], BF16, isOutput=False)
    dcomb_d = nc.declare_dram_parameter("dcomb", [128, 32], BF16, isOutput=False)
    o_d = nc.declare_dram_parameter("o", [BPC, C, N], BF16, isOutput=True)

    with tile.TileContext(nc) as tc, ExitStack() as ctx:
        const = ctx.enter_context(tc.tile_pool(name="const", bufs=1))
        stat = ctx.enter_context(tc.tile_pool(name="stat", bufs=1))
        io = ctx.enter_context(tc.tile_pool(name="io", bufs=2))
        wk1 = ctx.enter_context(tc.tile_pool(name="wk1", bufs=1))
        wk2 = ctx.enter_context(tc.tile_pool(name="wk2", bufs=2))
        qmp = ctx.enter_context(tc.tile_pool(name="qmp", bufs=1))
        small = ctx.enter_context(tc.tile_pool(name="small", bufs=2))
        dbcp = ctx.enter_context(tc.tile_pool(name="dbcp", bufs=2))
        ocp = ctx.enter_context(tc.tile_pool(name="ocp", bufs=1))
        pctx = ctx.enter_context(tc.tile_pool(name="pctx", bufs=5, space="PSUM"))
        pout = ctx.enter_context(tc.tile_pool(name="pout", bufs=2, space="PSUM"))
        psd = ctx.enter_context(tc.tile_pool(name="psd", bufs=1, space="PSUM"))
        dram = ctx.enter_context(tc.tile_pool(name="dram", bufs=2, space="DRAM"))

        def issue_inputs(b, split=False):
            ksr = io.tile([128, 8, C], BF16, tag="ksr", name="ksr")
            qnb = io.tile([128, 4, N], BF16, tag="qnb", name="qnb")
            vst = io.tile([128, 8, 4, 130], BF16, tag="vst", name="vst")
            if split:
                # batch 0: half-granularity k/v transfers so the first ctx
                # MMs start as soon as the first token chunks land
                def khalf(h):
                    ts = slice(512 * h, 512 * (h + 1))
                    nc.sync.dma_start(
                        ksr[:, 4 * h:4 * h + 4, :],
                        ks_d[b, ts].rearrange("(s r) c -> r s c", r=128))
                    nc.sync.dma_start(
                        vst[:, 4 * h:4 * h + 4],
                        vsp_d[b, ts].rearrange("(s r) p c -> r s p c", r=128))
                khalf(0)
                nc.sync.dma_start(
                    qnb[:], qn_d[b].rearrange("(p r) n -> r p n", r=128))
                khalf(1)
            else:
                nc.sync.dma_start(
                    ksr[:], ks_d[b].rearrange("(s r) c -> r s c", r=128))
                nc.sync.dma_start(
                    qnb[:], qn_d[b].rearrange("(p r) n -> r p n", r=128))
                nc.sync.dma_start(
                    vst[:], vsp_d[b].rearrange("(s r) p c -> r s p c", r=128))
            return qnb, ksr, vst

        # tiny constants first (they gate the k/v-side elementwise ops and
        # the serialized DMA device would otherwise park them behind 3MB of
        # input transfers); big coef tables after the first batch's inputs
        ptrs = const.tile([128, 4], F32, tag="ptrs", name="ptrs")
        cx = const.tile([128, 8], F32, tag="cx", name="cx")
        sx = const.tile([128, 8], F32, tag="sx", name="sx")
        dcomb = const.tile([128, 32], BF16, tag="dcomb", name="dcomb")
        nc.sync.dma_start(ptrs[:], ptrs_d[:])
        nc.sync.dma_start(cx[:], cx_d[:])
        nc.sync.dma_start(sx[:], sx_d[:])
        nc.sync.dma_start(dcomb[:], dcomb_d[:])

        first_in = issue_inputs(0, split=True)

        coefq = const.tile([128, 4, N], BF16, tag="coefq", name="coefq")
        coefD = const.tile([128, N], BF16, tag="coefD", name="coefD")
        nc.sync.dma_start(coefq[:], coefq_d[:])
        nc.sync.dma_start(coefD[:], coefD_d[:])

        # persistent zero-initialized tiles (written regions rewritten per
        # batch, zero regions never touched); double-buffered by parity
        L = {}
        kD = {}
        for par in range(2):
            for p in range(4):
                Lt = stat.tile([128, 2, 2, 128], BF16, tag=f"L{par}{p}",
                               name=f"L{par}{p}")
                nc.gpsimd.memset(Lt[:].rearrange("r a b e -> r (a b e)"), 0.0)
                L[par, p] = Lt
                kt = stat.tile([128, 32], BF16, tag=f"kD{par}{p}",
                               name=f"kD{par}{p}")
                nc.gpsimd.memset(kt[:], 0.0)
                kD[par, p] = kt

        def front(b, qnb, ksr, vst, split=False):
            """k/q-side elementwise prep for batch b. Ordered so Pool's qm
            work (needs qa) can start as early as possible."""
            # eps terms dropped: 1e-5 is 4-5 orders below bf16 noise and the
            # 2e-2 gate; dn folded into the cosy/siny ptrs
            kc = wk1.tile([128, 8, C], BF16, tag="kc", name="kc")
            ks2 = wk1.tile([128, 8, C], BF16, tag="ks2", name="ks2")
            for sh in ((slice(0, 4), slice(4, 8)) if split else (slice(0, 8),)):
                nc.vector.tensor_scalar(
                    kc[:, sh, :].rearrange("r s c -> r (s c)"),
                    ksr[:, sh, :].rearrange("r s c -> r (s c)"),
                    ptrs[:, 0:1], 0.0, MULT, MAX)
                nc.vector.tensor_scalar(
                    ks2[:, sh, :].rearrange("r s c -> r (s c)"),
                    ksr[:, sh, :].rearrange("r s c -> r (s c)"),
                    ptrs[:, 1:2], 0.0, MULT, MAX)
            qr = wk1.tile([128, 4, N], BF16, tag="qr", name="qr")
            nc.vector.tensor_scalar(
                qr[:].rearrange("r p n -> r (p n)"),
                qnb[:].rearrange("r p n -> r (p n)"), DN, 0.0, MULT, MAX)
            qa = wk2.tile([128, 4, N], BF16, tag="qa", name="qa")
            for p in range(4):
                nc.scalar.activation(qa[:, p, :], qr[:, p, :], COPY, bias=EPS)
            vCS = wk1.tile([128, 8, 4, 2, 130], BF16, tag="vCS", name="vCS")
            for s in range(8):
                nc.vector.tensor_scalar_mul(
                    vCS[:, s, :, 0, :], vst[:, s], cx[:, s:s + 1])
                nc.vector.tensor_scalar_mul(
                    vCS[:, s, :, 1, :], vst[:, s], sx[:, s:s + 1])
            return kc, ks2, qa, vCS

        def norm_and_store(st, split=False):
            b, ocs, dbc, onrm = st
            for p in range(4):
                for half in range(2):
                    hs = slice(512 * half, 512 * (half + 1))
                    nc.vector.tensor_tensor(
                        onrm[:, p, hs], ocs[p, half][:], dbc[p][:, hs], MULT)
                if split:
                    nc.sync.dma_start(
                        o_d[b, 128 * p:128 * (p + 1), :], onrm[:, p, :])
            if not split:
                nc.sync.dma_start(
                    o_d[b].rearrange("(p r) n -> r p n", r=128), onrm[:])

        fr = front(0, *first_in, split=True)
        pending = None
        for b in range(BPC):
            par = b % 2
            kc, ks2, qa, vCS = fr
            if b + 1 < BPC:
                nxt_in = issue_inputs(b + 1)
            if pending is not None:
                norm_and_store(pending)
                pending = None

            # ---- qm_j = qa * coefq_j (DVE planes / Pool planes) ----
            qm = {}
            for j in range(4):
                t = qmp.tile([128, 4, N], BF16, tag=f"qm{j}", name=f"qm{j}")
                qm[j] = t
                ndve = 3 if j <= 1 else 2
                cb = coefq[:, j, :]
                nc.vector.tensor_tensor(
                    t[:, 0:ndve, :],
                    qa[:, 0:ndve, :],
                    cb.unsqueeze(1).broadcast_to([128, ndve, N]), MULT)
                nc.gpsimd.tensor_tensor(
                    t[:, ndve:4, :],
                    qa[:, ndve:4, :],
                    cb.unsqueeze(1).broadcast_to([128, 4 - ndve, N]), MULT)

            # ---- ctx per pair: 2 psum tiles, 16 MMs; harvest L + kD ----
            for p in range(4):
                pA = pctx.tile([128, 2, 130], F32, tag="pc", name="pA")
                pB = pctx.tile([128, 2, 130], F32, tag="pc", name="pB")
                for s in range(8):
                    nc.tensor.matmul(
                        pA[:], kc[:, s, 128 * p:128 * (p + 1)], vCS[:, s, p],
                        start=(s == 0), stop=(s == 7))
                    nc.tensor.matmul(
                        pB[:], ks2[:, s, 128 * p:128 * (p + 1)], vCS[:, s, p],
                        start=(s == 0), stop=(s == 7))
                for st, pX in ((0, pA), (1, pB)):
                    for h in range(2):
                        rs = slice(64 * h, 64 * (h + 1))
                        nc.scalar.activation(
                            L[par, p][rs, st, :, rs], pX[rs, :, rs], COPY)
                        nc.scalar.activation(
                            kD[par, p][rs, 2 * st + 4 * h:2 * st + 4 * h + 2],
                            pX[rs, :, 128:129], COPY)

            # ---- D: sps = kD^T qa ; sd = sps*coefD ; dps = dcomb^T sd ----
            dinv = small.tile([8, N], BF16, tag="dinv", name="dinv")
            for half in range(2):
                hs = slice(512 * half, 512 * (half + 1))
                sps = psd.tile([128, 512], F32, tag="sps", name="sps")
                for p in range(4):
                    nc.tensor.matmul(
                        sps[32 * p:32 * (p + 1), :], kD[par, p][:],
                        qa[:, p, hs], start=True, stop=True,
                        tile_position=(0, 32 * p))
                sd = small.tile([128, 512], BF16, tag="sd", name="sd")
                nc.vector.tensor_tensor(sd[:], sps[:], coefD[:, hs], MULT)
                dps = psd.tile([32, 512], F32, tag="sps", name="dps")
                nc.tensor.matmul(dps[:], dcomb[:], sd[:], start=True, stop=True)
                with nc.allow_low_precision(reason="bf16 D_inv within tolerance"):
                    nc.vector.reciprocal(dinv[0:8, hs], dps[0:8, :])

            # broadcast D_inv rows to 64 partitions each via DRAM bounce
            dvd = dram.tile([8, N], BF16, tag="dvd", name="dvd")
            nc.sync.dma_start(dvd[:], dinv[:])
            dbc = {}
            for p in range(4):
                dbc[p] = dbcp.tile([128, N], BF16, tag=f"dbc{p}", name=f"dbc{p}")
                nc.sync.dma_start(
                    dbc[p][:],
                    dvd[2 * p:2 * p + 2, :]
                    .partition_broadcast(64).rearrange("r h n -> h r n"))

            # ---- out phase; PSUM drained promptly by ACT copies. The
            # normalize TTs are deferred past the next batch's front so the
            # DVE queue never blocks on the dbc DMA chain. ----
            onrm = small.tile([128, 4, N], BF16, tag="onrm", name="onrm")
            ocs = {}
            for p in range(4):
                for half in range(2):
                    hs = slice(512 * half, 512 * (half + 1))
                    ops = pout.tile([128, 512], F32, tag="ops", name="ops")
                    for j in range(4):
                        st, var = j // 2, j % 2
                        nc.tensor.matmul(
                            ops[:], L[par, p][:, st, var, :], qm[j][:, p, hs],
                            start=(j == 0), stop=(j == 3))
                    oc = ocp.tile([128, 512], BF16, tag=f"oc{p}{half}",
                                  name="oc")
                    nc.scalar.activation(oc[:], ops[:], COPY)
                    ocs[p, half] = oc

            if b + 1 < BPC:
                fr = front(b + 1, *nxt_in)
            pending = (b, ocs, dbc, onrm)
        norm_and_store(pending, split=True)

    _fix_waits(nc)
    return nc


_NC = None


def _get_nc():
    global _NC
    if _NC is None:
        _NC = build_nc()
    return _NC


def _host_prep(q, k, v, n_head, M):
    n_head = int(n_head)
    M = int(M)
    assert q.shape == (B, C, H, W) and n_head == NHEAD

    idx = np.arange(H, dtype=np.float32)
    freq = np.float32(pi / (2 * M))
    co, si = np.cos(idx * freq), np.sin(idx * freq)
    t = np.arange(N)
    x, y = t // 32, t % 32
    r = np.arange(128)

    dn32 = np.float32(DN)
    ptrs = np.stack([dn32 * co[r % 32], dn32 * si[r % 32],
                     0 * co[r % 32], 0 * si[r % 32]], axis=1).astype(np.float32)
    cx = np.stack([co[(128 * s + r) // 32] for s in range(8)],
                  axis=1).astype(np.float32)
    sx = np.stack([si[(128 * s + r) // 32] for s in range(8)],
                  axis=1).astype(np.float32)

    # branch order j: [cc, sc, cs, ss]
    coefB = np.stack([co[x] * co[y], si[x] * co[y],
                      co[x] * si[y], si[x] * si[y]]).astype(np.float32)
    coefq = np.broadcast_to(coefB[None], (128, 4, N)).astype(BFNP)
    coefD = np.zeros((128, N), np.float32)
    dcomb = np.zeros((128, 32), np.float32)
    for p in range(4):
        for h in range(2):
            for j in range(4):
                coefD[32 * p + 4 * h + j] = coefB[j]
                dcomb[32 * p + 4 * h + j, 2 * p + h] = 1.0
    coefD = coefD.astype(BFNP)
    dcomb = dcomb.astype(BFNP)

    qf = q.reshape(B, C, N).astype(BFNP)
    kf = np.ascontiguousarray(
        k.reshape(B, C, N).transpose(0, 2, 1)).astype(BFNP)
    vt = np.ascontiguousarray(
        v.reshape(B, C, N).transpose(0, 2, 1))          # [B, N, C] f32
    vsp = np.zeros((B, N, 4, 130), np.float32)
    vsp[..., :128] = vt.reshape(B, N, 4, 128)
    vsp[..., 128] = 1.0
    vsp = vsp.astype(BFNP)

    in_maps = []
    for core in range(NCORES):
        b0 = core * BPC
        in_maps.append({
            "qn": qf[b0:b0 + BPC], "ks": kf[b0:b0 + BPC],
            "vsp": vsp[b0:b0 + BPC],
            "ptrs": ptrs, "cx": cx, "sx": sx,
            "coefq": coefq, "coefD": coefD, "dcomb": dcomb,
        })
    return in_maps


def run(q, k, v, n_head=8, M=48, trace=False):
    nc = _get_nc()
    in_maps = _host_prep(q, k, v, n_head, M)
    res = run_bass_kernel_spmd(nc, in_maps, core_ids=list(range(NCORES)),
                               trace=trace)
    outs = []
    for core in range(NCORES):
        o = np.asarray(res.results[core]["o"]).astype(np.float32)
        outs.append(o.reshape(BPC, C, H, W))
    return np.concatenate(outs, axis=0), res


def kernel(q, k, v, n_head=8, M=48):
    out, _ = run(q, k, v, n_head, M)
    return out
